# revision 1
# baseline (speedup 1.0000x reference)
"""KAN layer kernel for 8 Trainium2 NeuronCores.

Math (reference):
    basis[b,i] = sum_h silu(x[b,i]*w1[i%K,h] + b1[i%K,h]) * w2[i%K,h] + b2[i%K]
    out[b,o]   = sum_i basis[b,i] * Wsum[o,i],   Wsum = W.sum(-1)   # [O,I]

Sharding: data-parallel over the input-feature axis I (16384 -> 8 x 2048).
Each core computes a partial out[64,1024] over its feature slice; host sums.

Per-core device program (memory-bound on reading its 42 MB W slice):
  - W arrives host-transposed as Wt[i,k,o]; the k-reduction happens *inside
    the DMA* via serial accum_op=add transfers (SDMA CCE), so Wsum[i,o]
    lands in SBUF with zero engine work and contraction (i) already on the
    partition axis -- no on-chip transposes anywhere.
  - basis is computed with i on partitions: ACT evaluates
    silu(w1*x+b1) with per-partition scale/bias vectors; DVE accumulates
    w2*silu(+b2) via fused scalar_tensor_tensor. Result acc[i,b] is directly
    the matmul lhsT.
  - 2 fp32 matmuls per i-tile accumulate into PSUM across all 16 i-tiles.
"""
import numpy as np

B, I, O, K, H = 64, 16384, 1024, 5, 16
NCORES = 8
IC = I // NCORES          # 2048 features per core
P = 128                   # partition tile
NT = IC // P              # 16 i-tiles per core
NB = B                    # 64
NO = O                    # 1024
PRW = 3 * H + 1           # packed param cols per i-tile: w1,b1,w2 (16 ea) + b2
CBW = NT * NB + NT * PRW  # const tile width: x block + param block

TRACE = False             # test.py sets True to capture an NTFF profile
LAST_RESULT = None


def _build():
    from contextlib import ExitStack
    from concourse import bacc, mybir, tile

    dt = mybir.dt.float32
    nc = bacc.Bacc("TRN2", target_bir_lowering=False, debug=False,
                   num_devices=NCORES)
    Wt = nc.declare_dram_parameter("Wt", [IC, K, NO], dt, isOutput=False)
    cbd = nc.declare_dram_parameter("cb", [P, CBW], dt, isOutput=False)
    out = nc.declare_dram_parameter("out", [NB, NO], dt, isOutput=True)

    with tile.TileContext(nc) as tc, ExitStack() as ctx:
        const = ctx.enter_context(tc.tile_pool(name="const", bufs=1))
        wpool = ctx.enter_context(tc.tile_pool(name="w", bufs=8))
        bpool = ctx.enter_context(tc.tile_pool(name="basis", bufs=16))
        spool = ctx.enter_context(tc.tile_pool(name="silu", bufs=3))
        opool = ctx.enter_context(tc.tile_pool(name="out", bufs=1))
        psum = ctx.enter_context(tc.tile_pool(name="psum", bufs=1, space="PSUM"))

        cb = const.tile([P, CBW], dt)
        nc.sync.dma_start(cb[:, :], cbd[:, :])

        ps0 = psum.tile([NB, 512], dt, tag="ps0")
        ps1 = psum.tile([NB, 512], dt, tag="ps1")

        # ---- basisT[i,b] for every i-tile (ACT/DVE only; no W dependency) ----
        accs = []
        for t in range(NT):
            xs = cb[:, t * NB:(t + 1) * NB]
            pb = NT * NB + t * PRW
            acc = bpool.tile([P, NB], dt)
            for h in range(H):
                st = spool.tile([P, NB], dt)
                nc.scalar.activation(
                    st[:, :], xs, mybir.ActivationFunctionType.Silu,
                    bias=cb[:, pb + H + h:pb + H + h + 1],
                    scale=cb[:, pb + h:pb + h + 1],
                )
                if h == 0:
                    # acc = w2[:,0]*silu + b2
                    nc.vector.tensor_scalar(
                        acc[:, :], st[:, :],
                        cb[:, pb + 2 * H:pb + 2 * H + 1],
                        cb[:, pb + 3 * H:pb + 3 * H + 1],
                        op0=mybir.AluOpType.mult, op1=mybir.AluOpType.add,
                    )
                else:
                    # acc = w2[:,h]*silu + acc
                    nc.vector.scalar_tensor_tensor(
                        acc[:, :], st[:, :],
                        cb[:, pb + 2 * H + h:pb + 2 * H + h + 1],
                        acc[:, :],
                        op0=mybir.AluOpType.mult, op1=mybir.AluOpType.add,
                    )
            accs.append(acc)

        # ---- Wsum[i,o] = sum_k Wt[i,k,o], reduced inside the DMA.
        # All SWDGE DMAs issue in program order from the one gpsimd
        # sequencer, and step k of a tile must wait for step k-1's
        # completion (~2us). Interleaving the chains of a window of tiles
        # keeps every wait pre-satisfied so the queue never stalls. ----
        WIN = 4
        wsums = [None] * NT
        for base in range(0, NT, WIN):
            grp = range(base, min(base + WIN, NT))
            for t in grp:
                wsums[t] = wpool.tile([P, NO], dt, tag="wsum", name=f"wsum{t}")
            for k in range(K):
                for t in grp:
                    nc.gpsimd.dma_start(
                        wsums[t][:, :], Wt[t * P:(t + 1) * P, k, :],
                        accum_op=(mybir.AluOpType.bypass if k == 0
                                  else mybir.AluOpType.add))

        # ---- partial matmuls: out[b,o] += basisT.T @ Wsum ----
        for t in range(NT):
            nc.tensor.matmul(ps0[:, :], accs[t][:, :], wsums[t][:, 0:512],
                             start=(t == 0), stop=(t == NT - 1))
            nc.tensor.matmul(ps1[:, :], accs[t][:, :], wsums[t][:, 512:1024],
                             start=(t == 0), stop=(t == NT - 1))

        out_sb = opool.tile([NB, NO], dt)
        nc.vector.tensor_copy(out_sb[:, 0:512], ps0[:, :])
        nc.vector.tensor_copy(out_sb[:, 512:1024], ps1[:, :])
        nc.sync.dma_start(out[:, :], out_sb[:, :])
    nc.compile()
    return nc


def kernel(x, w1, b1, w2, b2, W):
    global LAST_RESULT
    from concourse.bass_utils import run_bass_kernel_spmd

    x = np.asarray(x, dtype=np.float32)
    W = np.asarray(W, dtype=np.float32)
    w1 = np.asarray(w1, dtype=np.float32)
    b1 = np.asarray(b1, dtype=np.float32)
    w2 = np.asarray(w2, dtype=np.float32)
    b2 = np.asarray(b2, dtype=np.float32)

    # ---- host prep: W -> [I,K,O] (contraction-major layout for the PE) ----
    Wt_full = np.ascontiguousarray(W.reshape(O, I * K).T).reshape(I, K, O)

    idx = np.arange(I) % K
    w1e, b1e, w2e = w1[idx], b1[idx], w2[idx]          # [I,H]
    b2e = b2[idx][:, None]                             # [I,1]
    pr = np.concatenate([w1e, b1e, w2e, b2e], axis=1)  # [I, PRW]

    in_maps = []
    for c in range(NCORES):
        sl = slice(c * IC, (c + 1) * IC)
        # x slice, transposed to [i, b], then swizzled to SBUF layout [P, NT*NB]
        xt = np.ascontiguousarray(x[:, sl].T)          # [IC, NB]
        xt_sb = xt.reshape(NT, P, NB).transpose(1, 0, 2).reshape(P, NT * NB)
        pr_sb = pr[sl].reshape(NT, P, PRW).transpose(1, 0, 2).reshape(P, NT * PRW)
        cb = np.ascontiguousarray(
            np.concatenate([xt_sb, pr_sb], axis=1), dtype=np.float32)
        in_maps.append({"Wt": Wt_full[sl], "cb": cb})

    nc = _build()
    res = run_bass_kernel_spmd(nc, in_maps, list(range(NCORES)), trace=TRACE)
    LAST_RESULT = res
    out = np.zeros((B, O), dtype=np.float32)
    for c in range(NCORES):
        out += res.results[c]["out"]
    return out



# revision 2
# speedup vs baseline: 1.3971x; 1.3971x over previous
"""KAN layer kernel for 8 Trainium2 NeuronCores.

Math (reference):
    basis[b,i] = sum_h silu(x[b,i]*w1[i%K,h] + b1[i%K,h]) * w2[i%K,h] + b2[i%K]
    out[b,o]   = sum_i basis[b,i] * Wsum[o,i],   Wsum = W.sum(-1)   # [O,I]

Strategy (memory-bound on streaming W):
  - Features are permuted so they are sorted by k = i%K.  Each SBUF
    partition then holds NT features of a SINGLE k, so the per-feature MLP
    params (w1,b1,w2,b2) are per-partition vectors valid across the whole
    free axis -> the basis MLP runs as a handful of WIDE instructions
    per 128-feature tile instead of one narrow op per (tile,h):
      z[p,(h,b)] = x[p,b] * w1rep[p,(h,b)] + b1rep       (2 DVE tensor ops)
      s = silu(z)                                        (1 ACT op, N=1024)
      acc[p,b] = sum_h s*w2rep + b2                      (mult + add-tree)
    All in bf16 -> DVE 2x mode; well under the DMA roofline.
  - W is cast to bf16 on host (tolerance is 2e-2; measured ~1e-3) and
    streamed raw with plain HWDGE DMAs -- no accum_op (the baseline's
    DMA-side K-reduction did SBUF read-modify-write at ~half rate).  The
    K-reduction instead rides the PE's PSUM accumulation: 5x more
    matmuls, still far from Tensor roofline and fully hidden by DMA.
  - Data-parallel over features: core c takes 121 partitions x 17 slots
    of the k-sorted (padded) feature list; partial out[64,1024] summed on
    host.  W traffic/core: 2057 rows x 5 k x 1024 x 2B = 21.1 MB ->
    ~59us at the 358 GB/s per-core HBM limit.
"""
import numpy as np

B, I, O, K, H = 64, 16384, 1024, 5, 16
NCORES = 8
NT = 17                   # feature slots per partition (= i-tiles per core)
GP = 193                  # partitions per k-group (ceil(3277/17))
APC = 121                 # active partitions per core (8*121=968 >= 5*193)
NPART = NCORES * APC      # 968 partitions globally
P = 128

TRACE = False             # test.py sets True to capture an NTFF profile
LAST_RESULT = None


def _build():
    from contextlib import ExitStack
    from concourse import bacc, mybir, tile

    f32 = mybir.dt.float32
    bf16 = mybir.dt.bfloat16
    nc = bacc.Bacc("TRN2", target_bir_lowering=False, debug=False,
                   num_devices=NCORES)
    Wd = nc.declare_dram_parameter("Wd", [NT, APC, K * O], bf16, isOutput=False)
    xd = nc.declare_dram_parameter("xd", [P, NT * B], bf16, isOutput=False)
    prd = nc.declare_dram_parameter("prd", [P, 3 * H * B], bf16, isOutput=False)
    b2d = nc.declare_dram_parameter("b2d", [P, 1], f32, isOutput=False)
    out = nc.declare_dram_parameter("out", [B, O], f32, isOutput=True)

    HB = H * B            # 1024 free elems per tile: (h, b) h-major
    with tile.TileContext(nc) as tc, ExitStack() as ctx:
        const = ctx.enter_context(tc.tile_pool(name="const", bufs=1))
        wpool = ctx.enter_context(tc.tile_pool(name="w", bufs=5))
        zpool = ctx.enter_context(tc.tile_pool(name="z", bufs=2))
        spool = ctx.enter_context(tc.tile_pool(name="s", bufs=2))
        fpool = ctx.enter_context(tc.tile_pool(name="fold", bufs=2))
        apool = ctx.enter_context(tc.tile_pool(name="acc", bufs=NT))
        opool = ctx.enter_context(tc.tile_pool(name="out", bufs=1))
        psum = ctx.enter_context(tc.tile_pool(name="psum", bufs=1, space="PSUM"))

        # Constants ride the scalar HWDGE queue so the sync queue is a pure
        # W stream from t=0.
        xsb = const.tile([P, NT * B], bf16)
        nc.scalar.dma_start(xsb[:, :], xd[:, :])
        prsb = const.tile([P, 3 * HB], bf16)
        nc.scalar.dma_start(prsb[:, :], prd[:, :])
        b2sb = const.tile([P, 1], f32)
        nc.scalar.dma_start(b2sb[:, :], b2d[:, :])
        w1rep = prsb[:, 0:HB]
        b1rep = prsb[:, HB:2 * HB]
        w2rep = prsb[:, 2 * HB:3 * HB]

        ps0 = psum.tile([B, 512], f32, tag="ps0")
        ps1 = psum.tile([B, 512], f32, tag="ps1")

        # ---- basis acc[p,b] per i-tile (DVE/ACT only; no W dependency) ----
        accs = []
        for t in range(NT):
            xs = xsb[:, t * B:(t + 1) * B]
            xb = xs[:, None, :].to_broadcast((P, H, B))
            z = zpool.tile([P, HB], bf16, tag="z")
            z3 = z[:, :].rearrange("p (h b) -> p h b", h=H)
            w13 = w1rep.rearrange("p (h b) -> p h b", h=H)
            nc.vector.tensor_mul(z3, xb, w13)
            nc.vector.tensor_add(z[:, :], z[:, :], b1rep)
            s = spool.tile([P, HB], bf16, tag="s")
            nc.scalar.activation(s[:, :], z[:, :],
                                 mybir.ActivationFunctionType.Silu)
            sw = zpool.tile([P, HB], bf16, tag="sw")
            nc.vector.tensor_mul(sw[:, :], s[:, :], w2rep)
            f8 = fpool.tile([P, 8 * B], bf16, tag="f8")
            nc.vector.tensor_add(f8[:, :], sw[:, 0:8 * B], sw[:, 8 * B:HB])
            f4 = fpool.tile([P, 4 * B], bf16, tag="f4")
            nc.vector.tensor_add(f4[:, :], f8[:, 0:4 * B], f8[:, 4 * B:8 * B])
            f2 = fpool.tile([P, 2 * B], bf16, tag="f2")
            nc.vector.tensor_add(f2[:, :], f4[:, 0:2 * B], f4[:, 2 * B:4 * B])
            acc = apool.tile([P, B], bf16, tag="acc", name=f"acc{t}")
            nc.vector.scalar_tensor_tensor(
                acc[:, :], f2[:, 0:B], b2sb[:, 0:1], f2[:, B:2 * B],
                op0=mybir.AluOpType.add, op1=mybir.AluOpType.add)
            accs.append(acc)

        # ---- stream W (bf16, contiguous) and accumulate over (t, k) ----
        for t in range(NT):
            wt = wpool.tile([APC, K * O], bf16, tag="wt", name=f"wt{t}")
            nc.sync.dma_start(wt[:, :], Wd[t])
            for k in range(K):
                st = (t == 0 and k == 0)
                sp = (t == NT - 1 and k == K - 1)
                nc.tensor.matmul(ps0[:, :], accs[t][0:APC, :],
                                 wt[:, k * O:k * O + 512], start=st, stop=sp)
                nc.tensor.matmul(ps1[:, :], accs[t][0:APC, :],
                                 wt[:, k * O + 512:(k + 1) * O], start=st, stop=sp)

        out_sb = opool.tile([B, O], f32)
        nc.vector.tensor_copy(out_sb[:, 0:512], ps0[:, :])
        nc.vector.tensor_copy(out_sb[:, 512:1024], ps1[:, :])
        nc.scalar.dma_start(out[:, :], out_sb[:, :])
    nc.compile()
    return nc


def kernel(x, w1, b1, w2, b2, W):
    global LAST_RESULT
    import ml_dtypes
    from concourse.bass_utils import run_bass_kernel_spmd

    bf16 = ml_dtypes.bfloat16
    x = np.asarray(x, dtype=np.float32)
    W = np.asarray(W, dtype=np.float32)
    w1 = np.asarray(w1, dtype=np.float32)
    b1 = np.asarray(b1, dtype=np.float32)
    w2 = np.asarray(w2, dtype=np.float32)
    b2 = np.asarray(b2, dtype=np.float32)

    # ---- k-sorted feature permutation, padded so every partition holds
    # NT features of a single k ----
    kvec = np.arange(I) % K
    order = np.argsort(kvec, kind="stable")
    counts = [int(np.sum(kvec == k)) for k in range(K)]       # 3277x4, 3276
    plist = np.full(NPART * NT, -1, dtype=np.int64)
    off = 0
    for k in range(K):
        g0 = k * GP * NT
        plist[g0:g0 + counts[k]] = order[off:off + counts[k]]
        off += counts[k]
    feats = plist.reshape(NPART, NT)                          # [968, 17]
    Fidx = np.where(feats < 0, I, feats)                      # pad -> row I
    kpart = np.minimum(np.arange(NPART) // GP, K - 1)         # k per partition

    # ---- host prep ----
    xT = np.concatenate([np.ascontiguousarray(x.T),
                         np.zeros((1, B), np.float32)])       # [I+1, B]
    WT = np.ascontiguousarray(W.reshape(O, I * K).T).reshape(I, K, O)
    WTb = np.concatenate([WT, np.zeros((1, K, O), np.float32)]).astype(bf16)

    w1s, b1s, w2s = (np.repeat(a[kpart][:, :, None], B, axis=2)
                     .reshape(NPART, H * B) for a in (w1, b1, w2))
    b2s = b2[kpart].reshape(NPART, 1).astype(np.float32)

    in_maps = []
    for c in range(NCORES):
        rows = slice(c * APC, (c + 1) * APC)
        Fc = Fidx[rows]                                       # [121, 17]
        xg = np.zeros((P, NT * B), np.float32)
        xg[:APC] = xT[Fc].reshape(APC, NT * B)
        pr = np.zeros((P, 3 * H * B), np.float32)
        pr[:APC, 0:H * B] = w1s[rows]
        pr[:APC, H * B:2 * H * B] = b1s[rows]
        pr[:APC, 2 * H * B:3 * H * B] = w2s[rows]
        b2c = np.zeros((P, 1), np.float32)
        b2c[:APC] = b2s[rows]
        Wc = np.ascontiguousarray(
            WTb[Fc].transpose(1, 0, 2, 3).reshape(NT, APC, K * O))
        in_maps.append({
            "Wd": Wc,
            "xd": xg.astype(bf16),
            "prd": pr.astype(bf16),
            "b2d": b2c,
        })

    nc = _build()
    res = run_bass_kernel_spmd(nc, in_maps, list(range(NCORES)), trace=TRACE)
    LAST_RESULT = res
    out = np.zeros((B, O), dtype=np.float32)
    for c in range(NCORES):
        out += res.results[c]["out"]
    return out


# revision 3
# speedup vs baseline: 1.4171x; 1.0143x over previous
"""KAN layer kernel for 8 Trainium2 NeuronCores.

Math (reference):
    basis[b,i] = sum_h silu(x[b,i]*w1[i%K,h] + b1[i%K,h]) * w2[i%K,h] + b2[i%K]
    out[b,o]   = sum_i basis[b,i] * Wsum[o,i],   Wsum = W.sum(-1)   # [O,I]

Strategy (memory-bound on streaming W; per-core roofline ~21 MB / 358 GB/s
~ 59 us):
  - Features are permuted so they are sorted by k = i%K.  Each SBUF
    partition then holds NT features of a SINGLE k, so per-feature MLP
    params are per-partition values and the basis MLP runs as wide
    instructions over groups of 4 feature-tiles at once:
      z[p,(h,t,b)] = (x[p,(t,b)] * w1[p,h]) + b1[p,h]   (16 stt ops, DVE)
      s = silu(z)                                       (1 ACT op, N=4096)
      sw = s * w2rep                                    (1 DVE op)
      acc[p,(t,b)] = treefold_h(sw) + b2                (GPS+DVE adds)
    All bf16 (DVE 2x mode).  One leftover tile (17 = 1 + 4*4) runs the
    same path at G=1 and is streamed first so the PE has early work.
  - W is cast to bf16 on host (tolerance 2e-2, measured ~4e-3) and
    streamed with plain HWDGE DMAs on the sync queue -- const tensors go
    FIRST on the same queue so they are not stuck behind 5 MB of W
    packets on the shared SDMA engines.  The K-reduction rides the PE's
    PSUM accumulation (170 matmuls), hidden under the DMA stream; 8 W
    buffers decouple the DMA from the mm->sem->recycle latency loop.
  - Data-parallel over features: core c takes 121 partitions x 17 slots
    of the k-sorted (padded) feature list; partial out[64,1024] summed on
    host.
"""
import numpy as np

B, I, O, K, H = 64, 16384, 1024, 5, 16
NCORES = 8
NT = 17                   # feature slots per partition (= i-tiles per core)
G = 4                     # tiles per basis group (NT = 1 solo + 4 groups)
NG = 4
GP = 193                  # partitions per k-group (ceil(3277/17))
APC = 121                 # active partitions per core (8*121=968 >= 5*193)
NPART = NCORES * APC      # 968 partitions globally
P = 128

TRACE = False             # test.py sets True to capture an NTFF profile
LAST_RESULT = None


def _build():
    from contextlib import ExitStack
    from concourse import bacc, mybir, tile

    f32 = mybir.dt.float32
    bf16 = mybir.dt.bfloat16
    AT = mybir.ActivationFunctionType
    OP = mybir.AluOpType
    nc = bacc.Bacc("TRN2", target_bir_lowering=False, debug=False,
                   num_devices=NCORES)
    Wd = nc.declare_dram_parameter("Wd", [NT, APC, K * O], bf16, isOutput=False)
    xd = nc.declare_dram_parameter("xd", [P, NT * B], bf16, isOutput=False)
    # prd: b1rep [128,H*64] | w2repG [128,H*G*64]
    prd = nc.declare_dram_parameter("prd", [P, (H + H * G) * B], bf16,
                                    isOutput=False)
    # fpd: b2 [128,1] | w1 [128,H]  (fp32)
    fpd = nc.declare_dram_parameter("fpd", [P, 1 + H], f32, isOutput=False)
    out = nc.declare_dram_parameter("out", [B, O], f32, isOutput=True)

    GW = G * B                # 256: group row width (t,b)
    with tile.TileContext(nc) as tc, ExitStack() as ctx:
        const = ctx.enter_context(tc.tile_pool(name="const", bufs=1))
        wpool = ctx.enter_context(tc.tile_pool(name="w", bufs=8))
        zpool = ctx.enter_context(tc.tile_pool(name="z", bufs=2))
        spool = ctx.enter_context(tc.tile_pool(name="s", bufs=2))
        fpool = ctx.enter_context(tc.tile_pool(name="fold", bufs=2))
        apool = ctx.enter_context(tc.tile_pool(name="acc", bufs=NG + 1))
        opool = ctx.enter_context(tc.tile_pool(name="out", bufs=1))
        psum = ctx.enter_context(tc.tile_pool(name="psum", bufs=1, space="PSUM"))

        # Consts first on the sync HWDGE ring, ahead of the W stream.
        xsb = const.tile([P, NT * B], bf16)
        nc.sync.dma_start(xsb[:, :], xd[:, :])
        prsb = const.tile([P, (H + H * G) * B], bf16)
        nc.sync.dma_start(prsb[:, :], prd[:, :])
        fpsb = const.tile([P, 1 + H], f32)
        nc.sync.dma_start(fpsb[:, :], fpd[:, :])
        b1rep = prsb[:, 0:H * B]                       # [P,(h,b)]
        w2g = prsb[:, H * B:(H + H * G) * B]           # [P,(h,t,b)]
        b2v = fpsb[:, 0:1]
        w1f = fpsb[:, 1:1 + H]

        ps0 = psum.tile([B, 512], f32, tag="ps0")
        ps1 = psum.tile([B, 512], f32, tag="ps1")

        accs = [None] * NT   # per stream-slot: (tile_ap, col0)

        # ---- solo tile (stream slot 0): same path at G=1 ----
        xs0 = xsb[:, 0:B].rearrange("p (j b) -> p j b", j=1)
        z0 = zpool.tile([P, H * B], bf16, tag="z0")
        z03 = z0[:, :].rearrange("p (h b) -> p h b", h=H)
        for h in range(H):
            nc.vector.scalar_tensor_tensor(
                z03[:, h, :].rearrange("p (j b) -> p j b", j=1),
                xs0, w1f[:, h:h + 1],
                b1rep[:, h * B:(h + 1) * B][:, None, :].to_broadcast((P, 1, B)),
                op0=OP.mult, op1=OP.add)
        s0 = spool.tile([P, H * B], bf16, tag="s0")
        nc.scalar.activation(s0[:, :], z0[:, :], AT.Silu)
        sw0 = zpool.tile([P, H * B], bf16, tag="sw0")
        w2s = w2g.rearrange("p (h j b) -> p h j b", h=H, j=G)[:, :, 0, :]
        nc.vector.tensor_mul(sw0[:, :].rearrange("p (h b) -> p h b", h=H),
                             s0[:, :].rearrange("p (h b) -> p h b", h=H), w2s)
        f80 = fpool.tile([P, 8 * B], bf16, tag="f80")
        nc.vector.tensor_add(f80[:, :], sw0[:, 0:8 * B], sw0[:, 8 * B:16 * B])
        f40 = fpool.tile([P, 4 * B], bf16, tag="f40")
        nc.vector.tensor_add(f40[:, :], f80[:, 0:4 * B], f80[:, 4 * B:8 * B])
        f20 = fpool.tile([P, 2 * B], bf16, tag="f20")
        nc.vector.tensor_add(f20[:, :], f40[:, 0:2 * B], f40[:, 2 * B:4 * B])
        acc0 = apool.tile([P, B], bf16, tag="acc0")
        nc.vector.scalar_tensor_tensor(
            acc0[:, :], f20[:, 0:B], b2v, f20[:, B:2 * B],
            op0=OP.add, op1=OP.add)
        accs[0] = (acc0, 0)

        # ---- 4 groups of 4 tiles ----
        for g in range(NG):
            c0 = (1 + g * G) * B
            xg = xsb[:, c0:c0 + GW].rearrange("p (j b) -> p j b", j=G)
            zg = zpool.tile([P, H * GW], bf16, tag="zg", name=f"zg{g}")
            zg4 = zg[:, :].rearrange("p (h j b) -> p h j b", h=H, j=G)
            for h in range(H):
                nc.vector.scalar_tensor_tensor(
                    zg4[:, h, :, :], xg, w1f[:, h:h + 1],
                    b1rep[:, h * B:(h + 1) * B][:, None, :]
                    .to_broadcast((P, G, B)),
                    op0=OP.mult, op1=OP.add)
            sg = spool.tile([P, H * GW], bf16, tag="sg", name=f"sg{g}")
            nc.scalar.activation(sg[:, :], zg[:, :], AT.Silu)
            swg = zpool.tile([P, H * GW], bf16, tag="swg", name=f"swg{g}")
            nc.vector.tensor_mul(swg[:, :], sg[:, :], w2g)
            f8 = fpool.tile([P, 8 * GW], bf16, tag="f8", name=f"f8_{g}")
            nc.gpsimd.tensor_add(f8[:, :], swg[:, 0:8 * GW],
                                 swg[:, 8 * GW:16 * GW])
            f4 = fpool.tile([P, 4 * GW], bf16, tag="f4", name=f"f4_{g}")
            nc.gpsimd.tensor_add(f4[:, :], f8[:, 0:4 * GW], f8[:, 4 * GW:8 * GW])
            f2 = fpool.tile([P, 2 * GW], bf16, tag="f2", name=f"f2_{g}")
            nc.vector.tensor_add(f2[:, :], f4[:, 0:2 * GW], f4[:, 2 * GW:4 * GW])
            accg = apool.tile([P, GW], bf16, tag="accg", name=f"accg{g}")
            nc.vector.scalar_tensor_tensor(
                accg[:, :], f2[:, 0:GW], b2v, f2[:, GW:2 * GW],
                op0=OP.add, op1=OP.add)
            for tg in range(G):
                accs[1 + g * G + tg] = (accg, tg * B)

        # ---- stream W (bf16, contiguous) and accumulate over (t, k) ----
        for t in range(NT):
            wt = wpool.tile([APC, K * O], bf16, tag="wt", name=f"wt{t}")
            nc.sync.dma_start(wt[:, :], Wd[t])
            at, ac = accs[t]
            lhsT = at[0:APC, ac:ac + B]
            for k in range(K):
                st = (t == 0 and k == 0)
                sp = (t == NT - 1 and k == K - 1)
                nc.tensor.matmul(ps0[:, :], lhsT,
                                 wt[:, k * O:k * O + 512], start=st, stop=sp)
                nc.tensor.matmul(ps1[:, :], lhsT,
                                 wt[:, k * O + 512:(k + 1) * O], start=st, stop=sp)

        out_sb = opool.tile([B, O], f32)
        nc.scalar.copy(out_sb[:, 0:512], ps0[:, :])
        nc.vector.tensor_copy(out_sb[:, 512:1024], ps1[:, :])
        nc.sync.dma_start(out[:, :], out_sb[:, :])
    nc.compile()
    return nc


def kernel(x, w1, b1, w2, b2, W):
    global LAST_RESULT
    import ml_dtypes
    from concourse.bass_utils import run_bass_kernel_spmd

    bf16 = ml_dtypes.bfloat16
    x = np.asarray(x, dtype=np.float32)
    W = np.asarray(W, dtype=np.float32)
    w1 = np.asarray(w1, dtype=np.float32)
    b1 = np.asarray(b1, dtype=np.float32)
    w2 = np.asarray(w2, dtype=np.float32)
    b2 = np.asarray(b2, dtype=np.float32)

    # ---- k-sorted feature permutation, padded so every partition holds
    # NT features of a single k ----
    kvec = np.arange(I) % K
    order = np.argsort(kvec, kind="stable")
    counts = [int(np.sum(kvec == k)) for k in range(K)]       # 3277x4, 3276
    plist = np.full(NPART * NT, -1, dtype=np.int64)
    off = 0
    for k in range(K):
        g0 = k * GP * NT
        plist[g0:g0 + counts[k]] = order[off:off + counts[k]]
        off += counts[k]
    feats = plist.reshape(NPART, NT)                          # [968, 17]
    Fidx = np.where(feats < 0, I, feats)                      # pad -> row I
    kpart = np.minimum(np.arange(NPART) // GP, K - 1)         # k per partition

    # ---- host prep ----
    xT = np.concatenate([np.ascontiguousarray(x.T),
                         np.zeros((1, B), np.float32)])       # [I+1, B]
    WT = np.ascontiguousarray(W.reshape(O, I * K).T).reshape(I, K, O)
    WTb = np.concatenate([WT, np.zeros((1, K, O), np.float32)]).astype(bf16)

    b1rep = np.repeat(b1[kpart][:, :, None], B, axis=2).reshape(NPART, H * B)
    w2rep = np.repeat(w2[kpart][:, :, None], G * B, axis=2).reshape(
        NPART, H * G * B)
    w1f = w1[kpart]                                           # [NPART, H]
    b2f = b2[kpart].reshape(NPART, 1)

    in_maps = []
    for c in range(NCORES):
        rows = slice(c * APC, (c + 1) * APC)
        Fc = Fidx[rows]                                       # [121, 17]
        xg = np.zeros((P, NT * B), np.float32)
        xg[:APC] = xT[Fc].reshape(APC, NT * B)
        pr = np.zeros((P, (H + H * G) * B), np.float32)
        pr[:APC, 0:H * B] = b1rep[rows]
        pr[:APC, H * B:] = w2rep[rows]
        fp = np.zeros((P, 1 + H), np.float32)
        fp[:APC, 0:1] = b2f[rows]
        fp[:APC, 1:] = w1f[rows]
        Wc = np.ascontiguousarray(
            WTb[Fc].transpose(1, 0, 2, 3).reshape(NT, APC, K * O))
        in_maps.append({
            "Wd": Wc,
            "xd": xg.astype(bf16),
            "prd": pr.astype(bf16),
            "fpd": fp,
        })

    nc = _build()
    res = run_bass_kernel_spmd(nc, in_maps, list(range(NCORES)), trace=TRACE)
    LAST_RESULT = res
    out = np.zeros((B, O), dtype=np.float32)
    for c in range(NCORES):
        out += res.results[c]["out"]
    return out


# revision 4
# speedup vs baseline: 1.6569x; 1.1692x over previous
"""KAN layer kernel for 8 Trainium2 NeuronCores.

Math (reference):
    basis[b,i] = sum_h silu(x[b,i]*w1[i%K,h] + b1[i%K,h]) * w2[i%K,h] + b2[i%K]
    out[b,o]   = sum_i basis[b,i] * Wsum[o,i],   Wsum = W.sum(-1)   # [O,I]

Strategy (memory-bound on streaming W; per-core ~21 MB of bf16):
  - Features are permuted so they are sorted by k = i%K.  Each SBUF
    partition then holds NT features of a SINGLE k, so per-feature MLP
    params are per-partition vectors and the basis MLP runs as wide bf16
    2x-mode DVE ops (z = x*w1rep + b1rep per tile; silu / *w2 / h-fold
    tree over groups of 4 tiles), with the two biggest folds on GPSIMD
    and the final +b2 on the scalar engine (ACT bias) -- every engine
    stays far below the DMA roofline.
  - W is cast to bf16 on host (tolerance 2e-2, measured ~4e-3) and
    streamed with plain HWDGE DMAs alternating across BOTH rings
    (sync + scalar queues); consts go first on each ring.  The
    K-reduction rides the PE's PSUM accumulation (170 matmuls), hidden
    under the DMA stream; 8 W buffers decouple DMA from mm latency.
  - Data-parallel over features: core c takes 121 partitions x 17 slots
    of the k-sorted (padded) feature list; partial out[64,1024] summed on
    host.
"""
import numpy as np

B, I, O, K, H = 64, 16384, 1024, 5, 16
NCORES = 8
NT = 17                   # feature slots per partition (= i-tiles per core)
G = 4                     # tiles per basis group (NT = 1 solo + 4 groups)
NG = 4
GP = 193                  # partitions per k-group (ceil(3277/17))
APC = 121                 # active partitions per core (8*121=968 >= 5*193)
NPART = NCORES * APC      # 968 partitions globally
P = 128

TRACE = False             # test.py sets True to capture an NTFF profile
LAST_RESULT = None


def _build():
    from contextlib import ExitStack
    from concourse import bacc, mybir, tile

    f32 = mybir.dt.float32
    bf16 = mybir.dt.bfloat16
    AT = mybir.ActivationFunctionType
    OP = mybir.AluOpType
    nc = bacc.Bacc("TRN2", target_bir_lowering=False, debug=False,
                   num_devices=NCORES)
    Wd = nc.declare_dram_parameter("Wd", [NT, APC, K * O], bf16, isOutput=False)
    xd = nc.declare_dram_parameter("xd", [P, NT * B], bf16, isOutput=False)
    # prd: w1rep [P,H*64] | b1rep [P,H*64] | w2repG [P,H*G*64]
    prd = nc.declare_dram_parameter("prd", [P, (2 * H + H * G) * B], bf16,
                                    isOutput=False)
    fpd = nc.declare_dram_parameter("fpd", [P, 1], f32, isOutput=False)
    out = nc.declare_dram_parameter("out", [B, O], f32, isOutput=True)

    HB = H * B                # 1024
    GW = G * B                # 256: group row width (t,b)
    with tile.TileContext(nc) as tc, ExitStack() as ctx:
        const = ctx.enter_context(tc.tile_pool(name="const", bufs=1))
        wpool = ctx.enter_context(tc.tile_pool(name="w", bufs=8))
        zpool = ctx.enter_context(tc.tile_pool(name="z", bufs=2))
        spool = ctx.enter_context(tc.tile_pool(name="s", bufs=2))
        fpool = ctx.enter_context(tc.tile_pool(name="fold", bufs=2))
        apool = ctx.enter_context(tc.tile_pool(name="acc", bufs=NG + 1))
        opool = ctx.enter_context(tc.tile_pool(name="out", bufs=1))
        psum = ctx.enter_context(tc.tile_pool(name="psum", bufs=1, space="PSUM"))

        # Consts first on each HWDGE ring, ahead of the W stream.
        xsb = const.tile([P, NT * B], bf16)
        nc.sync.dma_start(xsb[:, :], xd[:, :])
        fpsb = const.tile([P, 1], f32)
        nc.sync.dma_start(fpsb[:, :], fpd[:, :])
        prsb = const.tile([P, (2 * H + H * G) * B], bf16)
        nc.scalar.dma_start(prsb[:, :], prd[:, :])
        w1rep = prsb[:, 0:HB]                          # [P,(h,b)]
        b1rep = prsb[:, HB:2 * HB]
        w2g = prsb[:, 2 * HB:2 * HB + H * GW]          # [P,(h,t,b)]
        b2v = fpsb[:, 0:1]

        ps0 = psum.tile([B, 512], f32, tag="ps0")
        ps1 = psum.tile([B, 512], f32, tag="ps1")

        accs = [None] * NT   # per stream-slot: (tile_ap, col0)
        w13 = w1rep.rearrange("p (h b) -> p h b", h=H)

        def basis_tile(xs, z, s):
            """z,s: [P,HB] tiles; xs: [P,B] slice -> silu(x*w1+b1) in s."""
            xb = xs[:, None, :].to_broadcast((P, H, B))
            nc.vector.tensor_mul(z[:, :].rearrange("p (h b) -> p h b", h=H),
                                 xb, w13)
            nc.vector.tensor_add(z[:, :], z[:, :], b1rep)
            nc.scalar.activation(s[:, :], z[:, :], AT.Silu)

        # ---- solo tile (stream slot 0) ----
        z0 = zpool.tile([P, HB], bf16, tag="z0")
        s0 = spool.tile([P, HB], bf16, tag="s0")
        basis_tile(xsb[:, 0:B], z0, s0)
        sw0 = zpool.tile([P, HB], bf16, tag="sw0")
        w2s = w2g.rearrange("p (h j b) -> p h j b", h=H, j=G)[:, :, 0, :]
        nc.vector.tensor_mul(sw0[:, :].rearrange("p (h b) -> p h b", h=H),
                             s0[:, :].rearrange("p (h b) -> p h b", h=H), w2s)
        f80 = fpool.tile([P, 8 * B], bf16, tag="f80")
        nc.vector.tensor_add(f80[:, :], sw0[:, 0:8 * B], sw0[:, 8 * B:16 * B])
        f40 = fpool.tile([P, 4 * B], bf16, tag="f40")
        nc.vector.tensor_add(f40[:, :], f80[:, 0:4 * B], f80[:, 4 * B:8 * B])
        f20 = fpool.tile([P, 2 * B], bf16, tag="f20")
        nc.vector.tensor_add(f20[:, :], f40[:, 0:2 * B], f40[:, 2 * B:4 * B])
        ap0 = fpool.tile([P, B], bf16, tag="ap0")
        nc.vector.tensor_add(ap0[:, :], f20[:, 0:B], f20[:, B:2 * B])
        acc0 = apool.tile([P, B], bf16, tag="acc0")
        nc.scalar.activation(acc0[:, :], ap0[:, :], AT.Identity, bias=b2v)
        accs[0] = (acc0, 0)

        # ---- 4 groups of 4 tiles ----
        for g in range(NG):
            c0 = (1 + g * G) * B
            zs, ss = [], []
            for j in range(G):
                z = zpool.tile([P, HB], bf16, tag=f"zg{j}", name=f"z{g}_{j}")
                s = spool.tile([P, HB], bf16, tag=f"sg{j}", name=f"s{g}_{j}")
                basis_tile(xsb[:, c0 + j * B:c0 + (j + 1) * B], z, s)
                zs.append(z)
                ss.append(s)
            # sw for the whole group in one wide op: s tiles are separate
            # buffers, so multiply per tile into one grouped sw buffer.
            swg = zpool.tile([P, H * GW], bf16, tag="swg", name=f"swg{g}")
            sw4 = swg[:, :].rearrange("p (h j b) -> p h j b", h=H, j=G)
            w24 = w2g.rearrange("p (h j b) -> p h j b", h=H, j=G)
            for j in range(G):
                nc.vector.tensor_mul(
                    sw4[:, :, j, :],
                    ss[j][:, :].rearrange("p (h b) -> p h b", h=H),
                    w24[:, :, j, :])
            f8 = fpool.tile([P, 8 * GW], bf16, tag="f8", name=f"f8_{g}")
            nc.gpsimd.tensor_add(f8[:, :], swg[:, 0:8 * GW],
                                 swg[:, 8 * GW:16 * GW])
            f4 = fpool.tile([P, 4 * GW], bf16, tag="f4", name=f"f4_{g}")
            nc.gpsimd.tensor_add(f4[:, :], f8[:, 0:4 * GW], f8[:, 4 * GW:8 * GW])
            f2 = fpool.tile([P, 2 * GW], bf16, tag="f2", name=f"f2_{g}")
            nc.vector.tensor_add(f2[:, :], f4[:, 0:2 * GW], f4[:, 2 * GW:4 * GW])
            apg = fpool.tile([P, GW], bf16, tag="apg", name=f"apg{g}")
            nc.vector.tensor_add(apg[:, :], f2[:, 0:GW], f2[:, GW:2 * GW])
            accg = apool.tile([P, GW], bf16, tag="accg", name=f"accg{g}")
            nc.scalar.activation(accg[:, :], apg[:, :], AT.Identity, bias=b2v)
            for tg in range(G):
                accs[1 + g * G + tg] = (accg, tg * B)

        # ---- stream W on both HWDGE rings, accumulate over (t, k) ----
        for t in range(NT):
            wt = wpool.tile([APC, K * O], bf16, tag="wt", name=f"wt{t}")
            eng = nc.sync if t % 2 == 0 else nc.scalar
            eng.dma_start(wt[:, :], Wd[t])
            at, ac = accs[t]
            lhsT = at[0:APC, ac:ac + B]
            for k in range(K):
                st = (t == 0 and k == 0)
                sp = (t == NT - 1 and k == K - 1)
                nc.tensor.matmul(ps0[:, :], lhsT,
                                 wt[:, k * O:k * O + 512], start=st, stop=sp)
                nc.tensor.matmul(ps1[:, :], lhsT,
                                 wt[:, k * O + 512:(k + 1) * O], start=st, stop=sp)

        out_sb = opool.tile([B, O], f32)
        nc.scalar.copy(out_sb[:, 0:512], ps0[:, :])
        nc.vector.tensor_copy(out_sb[:, 512:1024], ps1[:, :])
        nc.sync.dma_start(out[:, :], out_sb[:, :])
    nc.compile()
    return nc


def kernel(x, w1, b1, w2, b2, W):
    global LAST_RESULT
    import ml_dtypes
    from concourse.bass_utils import run_bass_kernel_spmd

    bf16 = ml_dtypes.bfloat16
    x = np.asarray(x, dtype=np.float32)
    W = np.asarray(W, dtype=np.float32)
    w1 = np.asarray(w1, dtype=np.float32)
    b1 = np.asarray(b1, dtype=np.float32)
    w2 = np.asarray(w2, dtype=np.float32)
    b2 = np.asarray(b2, dtype=np.float32)

    # ---- k-sorted feature permutation, padded so every partition holds
    # NT features of a single k ----
    kvec = np.arange(I) % K
    order = np.argsort(kvec, kind="stable")
    counts = [int(np.sum(kvec == k)) for k in range(K)]       # 3277x4, 3276
    plist = np.full(NPART * NT, -1, dtype=np.int64)
    off = 0
    for k in range(K):
        g0 = k * GP * NT
        plist[g0:g0 + counts[k]] = order[off:off + counts[k]]
        off += counts[k]
    feats = plist.reshape(NPART, NT)                          # [968, 17]
    Fidx = np.where(feats < 0, I, feats)                      # pad -> row I
    kpart = np.minimum(np.arange(NPART) // GP, K - 1)         # k per partition

    # ---- host prep ----
    xT = np.concatenate([np.ascontiguousarray(x.T),
                         np.zeros((1, B), np.float32)])       # [I+1, B]
    WT = np.ascontiguousarray(W.reshape(O, I * K).T).reshape(I, K, O)
    WTb = np.concatenate([WT, np.zeros((1, K, O), np.float32)]).astype(bf16)

    w1rep = np.repeat(w1[kpart][:, :, None], B, axis=2).reshape(NPART, H * B)
    b1rep = np.repeat(b1[kpart][:, :, None], B, axis=2).reshape(NPART, H * B)
    w2rep = np.repeat(w2[kpart][:, :, None], G * B, axis=2).reshape(
        NPART, H * G * B)
    b2f = b2[kpart].reshape(NPART, 1)

    in_maps = []
    for c in range(NCORES):
        rows = slice(c * APC, (c + 1) * APC)
        Fc = Fidx[rows]                                       # [121, 17]
        xg = np.zeros((P, NT * B), np.float32)
        xg[:APC] = xT[Fc].reshape(APC, NT * B)
        pr = np.zeros((P, (2 * H + H * G) * B), np.float32)
        pr[:APC, 0:H * B] = w1rep[rows]
        pr[:APC, H * B:2 * H * B] = b1rep[rows]
        pr[:APC, 2 * H * B:] = w2rep[rows]
        fp = np.zeros((P, 1), np.float32)
        fp[:APC] = b2f[rows]
        Wc = np.ascontiguousarray(
            WTb[Fc].transpose(1, 0, 2, 3).reshape(NT, APC, K * O))
        in_maps.append({
            "Wd": Wc,
            "xd": xg.astype(bf16),
            "prd": pr.astype(bf16),
            "fpd": fp,
        })

    nc = _build()
    res = run_bass_kernel_spmd(nc, in_maps, list(range(NCORES)), trace=TRACE)
    LAST_RESULT = res
    out = np.zeros((B, O), dtype=np.float32)
    for c in range(NCORES):
        out += res.results[c]["out"]
    return out
